# revision 14
# baseline (speedup 1.0000x reference)
"""Trainium2 Bass kernel for MBart GQA attention.

Problem: B=2, T=2048, E=1024, 16 q-heads, 4 kv-heads, head_dim 64.
Sharding: 8 cores = 2 batches x 4 kv-heads (tensor-parallel over head
groups). Each core computes, for its (batch b, kv-head k):
  - q/k/v projections for its 4 q-heads (q channels k*256:(k+1)*256,
    k/v channels k*64:(k+1)*64), with q pre-scaled by D**-0.5,
  - attention in transposed layout: s^T[tk,tq] = (k_tile)^T-matmuls,
    exp on ScalarE, then out^T = [1|v]^T @ e^T so row 0 of the AV
    accumulator is the softmax denominator,
  - normalization (reciprocal + partition-broadcast + multiply),
  - its partial out-projection  ctx_k @ Wo[:, k*256:(k+1)*256].T.

Host<->device traffic is minimized with on-device collectives:
  - hidden_states are uploaded E-sliced (each core gets 1/4 of x^T rows
    for its batch) and AllGathered across the 4-core batch group, so
    each x element crosses the host link once.
  - the big weights (Wq/Wk/Wv/Wo slices for kv-head k) are packed into
    one bf16 blob shared by the two cores {k, k+4} (same weights, both
    batches); each uploads half and a pair AllGather reconstructs it.
  - the 4 per-core out-projection partials of a batch are summed on
    device with a ReduceScatter(add), so each core downloads only its
    disjoint [T/4, E] slice of the final output (fp16); host adds bo.

All matmuls bf16 inputs with fp32 PSUM accumulation.
"""

import os
import sys

for _p in ("/opt/trn_rl_repo", "/root/.axon_site/_ro/trn_rl_repo"):
    if os.path.isdir(_p) and _p not in sys.path:
        sys.path.insert(0, _p)

import numpy as np
import ml_dtypes

import concourse.bass as bass
import concourse.mybir as mybir
import concourse.tile as tile
from concourse import bacc
from concourse.bass_utils import run_bass_kernel_spmd

B, T, E = 2, 2048, 1024
H, KVH = 16, 4
D = E // H            # 64
G = H // KVH          # 4 q-heads per kv-head (= per core)
SCALE = D ** -0.5
NCORES = 8

BF16 = mybir.dt.bfloat16
F16 = mybir.dt.float16
F32 = mybir.dt.float32
NPBF16 = ml_dtypes.bfloat16

ROW_PACK = True  # pack two K=64 score matmuls into the 128x128 PE array

# packed weight blob (per kv-head): wq [128,8,G*D] | wkv [128,8,2*D] | wo [128,2,E]
WQ_N = 128 * 8 * G * D        # 262144
WKV_N = 128 * 8 * 2 * D       # 131072
WO_N = 128 * 2 * E            # 262144
WBLOB = WQ_N + WKV_N + WO_N   # 655360


def build_nc(t=T):
    """Build the per-core Bass program (SPMD: same program, per-core data)."""
    assert t % 512 == 0
    ch = min(512, t)          # free-dim chunk for matmuls / psum banks
    ntqc = t // ch            # number of T chunks
    tkt = t // 128            # number of 128-row key tiles
    ne = E // 128             # 8 contraction tiles for projections

    nc = bacc.Bacc(None, target_bir_lowering=False)

    xs_d = nc.declare_dram_parameter("xs", [E // 4, t], mybir.dt.int8, isOutput=False)
    xsc_d = nc.declare_dram_parameter("xsc", [128, E // 128], F32, isOutput=False)
    wh_d = nc.declare_dram_parameter("wh", [WBLOB // 2], BF16, isOutput=False)
    bq_d = nc.declare_dram_parameter("bq", [128, 2], F32, isOutput=False)
    bkv_d = nc.declare_dram_parameter("bkv", [128, 1], F32, isOutput=False)
    id_d = nc.declare_dram_parameter("ident", [64, 64], BF16, isOutput=False)
    y_d = nc.declare_dram_parameter("y", [t // 4, E], mybir.dt.uint8, isOutput=True)
    ys_d = nc.declare_dram_parameter("ys", [128, t // 512], F32, isOutput=True)

    with tile.TileContext(nc) as tc:
        with (
            tc.tile_pool(name="const", bufs=1) as const,
            tc.tile_pool(name="work", bufs=2) as work,
            tc.tile_pool(name="dram", bufs=1, space="DRAM") as dram,
        ):
            # ---- collectives: gather x (batch group) + weights (head pair) ----
            xsb = dram.tile([E // 4, t], mybir.dt.int8)
            whb = dram.tile([WBLOB // 2], BF16)
            xg = dram.tile([E, t], mybir.dt.int8)
            wb = dram.tile([WBLOB], BF16)
            yp = dram.tile([t, E], F32)
            yr = dram.tile([t // 4, E], F32)
            nc.sync.dma_start(xsb[:], xs_d[:])
            nc.sync.dma_start(whb[:], wh_d[:])
            nc.gpsimd.collective_compute(
                "AllGather", mybir.AluOpType.bypass,
                replica_groups=[[0, 1, 2, 3], [4, 5, 6, 7]],
                ins=[xsb[:].opt()], outs=[xg[:].opt()],
            )
            nc.gpsimd.collective_compute(
                "AllGather", mybir.AluOpType.bypass,
                replica_groups=[[0, 4], [1, 5], [2, 6], [3, 7]],
                ins=[whb[:].opt()], outs=[wb[:].opt()],
            )

            # ---- static SBUF tensors ----
            x8_sb = const.tile([128, ne, t], mybir.dt.int8)
            xsc_sb = const.tile([128, ne], F32)
            xT_sb = const.tile([128, ne, t], BF16)
            wq_sb = const.tile([128, ne, G * D], BF16)
            wkv_sb = const.tile([128, ne, 2 * D], BF16)
            wo_sb = const.tile([128, 2, E], BF16)
            bq_sb = const.tile([128, 2], F32)
            bkv_sb = const.tile([128, 1], F32)
            id_sb = const.tile([64, 64], BF16)
            zb_sb = const.tile([128, 1], F32)        # zero bias for Exp
            on_sb = const.tile([1, 1 + D], F32)      # ones row for bcast mm
            qTd_sb = const.tile([128, G, t], BF16)   # q^T per head, dup halves
            kT2_sb = const.tile([128, t], BF16)      # k^T dup in both halves
            vT_sb = const.tile([64, t], BF16)        # v^T at partitions 0-63
            kvn_sb = const.tile([128, t], BF16)      # k^T / v^T proj staging
            va_sb = const.tile([128, tkt, 1 + D], BF16)  # [1|v] per tk tile
            cT_sb = const.tile([128, 2, t], BF16)    # ctx^T (4 heads = 256 ch)

            nc.sync.dma_start(x8_sb[:], xg[:].rearrange("(e p) t -> p e t", p=128))
            nc.sync.dma_start(xsc_sb[:], xsc_d[:])
            # dequantize: x^T[channel, t] = int8 * per-channel scale -> bf16
            for e in range(ne):
                nc.vector.tensor_scalar(
                    xT_sb[:, e, :], x8_sb[:, e, :], xsc_sb[:, e:e + 1], None,
                    op0=mybir.AluOpType.mult,
                )
            nc.sync.dma_start(
                wq_sb[:], wb[0:WQ_N].rearrange("(p e d) -> p e d", p=128, e=ne)
            )
            nc.sync.dma_start(
                wkv_sb[:],
                wb[WQ_N:WQ_N + WKV_N].rearrange("(p e d) -> p e d", p=128, e=ne),
            )
            nc.sync.dma_start(
                wo_sb[:],
                wb[WQ_N + WKV_N:WBLOB].rearrange("(p c e) -> p c e", p=128, c=2),
            )
            nc.sync.dma_start(bq_sb[:], bq_d[:])
            nc.sync.dma_start(bkv_sb[:], bkv_d[:])
            nc.sync.dma_start(id_sb[:], id_d[:])
            nc.gpsimd.memset(zb_sb[:], 0.0)
            nc.gpsimd.memset(va_sb[:, :, 0], 1.0)
            nc.gpsimd.memset(on_sb[:], 1.0)

            # ---- projections: q^T [256,t], kv^T [128,t] (E-contraction) ----
            with tc.tile_pool(name="psum_proj", bufs=2, space="PSUM") as pp:
                for c in range(ntqc):
                    cs = slice(c * ch, (c + 1) * ch)
                    for w in range(3):
                        ps = pp.tile([128, ch], F32, tag="pp")
                        for e in range(ne):
                            lhsT = (
                                wq_sb[:, e, w * 128:(w + 1) * 128]
                                if w < 2
                                else wkv_sb[:, e, :]
                            )
                            nc.tensor.matmul(
                                ps[:],
                                lhsT,
                                xT_sb[:, e, cs],
                                start=(e == 0),
                                stop=(e == ne - 1),
                            )
                        ident_f = mybir.ActivationFunctionType.Identity
                        if w < 2:
                            # heads 2w (rows 0-63) and 2w+1 (rows 64-127)
                            nc.scalar.activation(
                                qTd_sb[0:64, 2 * w, cs], ps[0:64, :],
                                ident_f, bias=bq_sb[0:64, w:w + 1],
                            )
                            nc.scalar.activation(
                                qTd_sb[64:128, 2 * w + 1, cs], ps[64:128, :],
                                ident_f, bias=bq_sb[64:128, w:w + 1],
                            )
                        else:
                            nc.scalar.activation(
                                kvn_sb[0:64, cs], ps[0:64, :],
                                ident_f, bias=bkv_sb[0:64, :],
                            )
                            nc.scalar.activation(
                                kvn_sb[64:128, cs], ps[64:128, :],
                                ident_f, bias=bkv_sb[64:128, :],
                            )

                # duplicate q per head into both partition halves (row tiling
                # tile T8 reads both operands from partitions 64-127)
                nc.gpsimd.dma_start(qTd_sb[64:128, 0, :], qTd_sb[0:64, 0, :])
                nc.gpsimd.dma_start(qTd_sb[0:64, 1, :], qTd_sb[64:128, 1, :])
                nc.gpsimd.dma_start(qTd_sb[64:128, 2, :], qTd_sb[0:64, 2, :])
                nc.gpsimd.dma_start(qTd_sb[0:64, 3, :], qTd_sb[64:128, 3, :])
                nc.gpsimd.dma_start(kT2_sb[0:64, :], kvn_sb[0:64, :])
                nc.gpsimd.dma_start(kT2_sb[64:128, :], kvn_sb[0:64, :])
                nc.gpsimd.dma_start(vT_sb[:, :], kvn_sb[64:128, :])

                # transpose v^T [64,t] -> v [t,64] into va_sb[:, i, 1:65]
                for i in range(tkt):
                    tp = pp.tile([128, 64], BF16, tag="tp")
                    nc.tensor.transpose(
                        tp[:], vT_sb[:, i * 128:(i + 1) * 128], id_sb[:]
                    )
                    nc.vector.tensor_copy(va_sb[:, i, 1:1 + 64], tp[:])

            # ---- attention + out-projection ----
            psum_attn_cm = tc.tile_pool(name="psum_attn", bufs=1, space="PSUM")
            psum_attn = psum_attn_cm.__enter__()
            for c in range(ntqc):
                cs = slice(c * ch, (c + 1) * ch)
                for h in range(G):
                    sT = work.tile([128, tkt * ch], F32, tag="sT")
                    eT = work.tile([128, tkt * ch], BF16, tag="eT")
                    # scores^T: s[tk, tq] for each 128-row key tile
                    if ROW_PACK:
                        for p in range(tkt // 2):
                            psA = psum_attn.tile([128, ch], F32, tag="sc", bufs=4)
                            psB = psum_attn.tile([128, ch], F32, tag="sc", bufs=4)
                            nc.tensor.matmul(
                                psA[:],
                                kT2_sb[0:64, (2 * p) * 128:(2 * p + 1) * 128],
                                qTd_sb[0:64, h, cs],
                                start=True, stop=True,
                                tile_position=(0, 0),
                            )
                            nc.tensor.matmul(
                                psB[:],
                                kT2_sb[64:128, (2 * p + 1) * 128:(2 * p + 2) * 128],
                                qTd_sb[64:128, h, cs],
                                start=True, stop=True,
                                tile_position=(64, 0),
                            )
                            nc.vector.tensor_copy(
                                sT[:, (2 * p) * ch:(2 * p + 1) * ch], psA[:]
                            )
                            nc.vector.tensor_copy(
                                sT[:, (2 * p + 1) * ch:(2 * p + 2) * ch], psB[:]
                            )
                    else:
                        for p in range(tkt):
                            psA = psum_attn.tile([128, ch], F32, tag="sc", bufs=4)
                            nc.tensor.matmul(
                                psA[:],
                                kT2_sb[0:64, p * 128:(p + 1) * 128],
                                qTd_sb[0:64, h, cs],
                                start=True, stop=True,
                            )
                            nc.vector.tensor_copy(
                                sT[:, p * ch:(p + 1) * ch], psA[:]
                            )

                    # exp over the whole [128, tkt*ch] block in one ACT op
                    nc.scalar.activation(
                        eT[:], sT[:], mybir.ActivationFunctionType.Exp,
                        bias=zb_sb[:],
                    )

                    # out^T accumulate: [1|v]^T @ e^T -> [65, ch]
                    po = psum_attn.tile([1 + D, ch], F32, tag="av", bufs=2)
                    for p in range(tkt):
                        nc.tensor.matmul(
                            po[:],
                            va_sb[:, p, :],
                            eT[:, p * ch:(p + 1) * ch],
                            start=(p == 0),
                            stop=(p == tkt - 1),
                        )

                    # normalize: rows 1-64 divided by row 0 (softmax denom)
                    recip = work.tile([1, ch], F32, tag="recip")
                    nc.vector.reciprocal(recip[:], po[0:1, :])
                    # broadcast recip across partitions: ones[1,65]^T @ recip
                    bc = psum_attn.tile([1 + D, ch], F32, tag="sc", bufs=4)
                    nc.tensor.matmul(bc[:], on_sb[:], recip[:],
                                     start=True, stop=True)
                    bc_sb = work.tile([1 + D, ch], F32, tag="bc_sb")
                    nc.vector.tensor_copy(bc_sb[:], bc[:])
                    cstg = work.tile([1 + D, ch], BF16, tag="cstg")
                    nc.vector.tensor_mul(cstg[:], po[:], bc_sb[:])
                    nc.gpsimd.dma_start(
                        cT_sb[(h % 2) * 64:(h % 2) * 64 + 64, h // 2, cs],
                        cstg[1:1 + 64, :],
                    )

                # out-projection for this T chunk (all 4 heads done)
                for tqt in range(ch // 128):
                    tq0 = c * ch + tqt * 128
                    for nh in range(E // 512):
                        py = psum_attn.tile([128, 512], F32, tag="yp", bufs=2)
                        for ct in range(2):
                            nc.tensor.matmul(
                                py[:],
                                cT_sb[:, ct, tq0:tq0 + 128],
                                wo_sb[:, ct, nh * 512:(nh + 1) * 512],
                                start=(ct == 0),
                                stop=(ct == 1),
                            )
                        ysb = work.tile([128, 512], F32, tag="ysb")
                        nc.vector.tensor_copy(ysb[:], py[:])
                        nc.sync.dma_start(
                            yp[tq0:tq0 + 128, nh * 512:(nh + 1) * 512], ysb[:]
                        )
            psum_attn_cm.__exit__(None, None, None)

            # ---- sum partials across the batch group, download 1/4 slice ----
            nc.gpsimd.collective_compute(
                "ReduceScatter", mybir.AluOpType.add,
                replica_groups=[[0, 1, 2, 3], [4, 5, 6, 7]],
                ins=[yp[:].opt()], outs=[yr[:].opt()],
            )
            # uint8 quantization with per-row scales: row r of yr maps to
            # (partition p, group g) with r = g*128 + p.  u = v*126/absmax
            # + 128.5, dequantized on host as (u - 128) * absmax/126.
            ng = t // 512
            yt = work.tile([128, ng, E], F32, tag="sT")
            nc.sync.dma_start(yt[:], yr[:].rearrange("(g p) e -> p g e", p=128))
            yab = work.tile([128, ng], F32, tag="yab")
            nc.vector.tensor_reduce(
                yab[:], yt[:], mybir.AxisListType.X, mybir.AluOpType.max,
                apply_absolute_value=True,
            )
            ysc = work.tile([128, ng], F32, tag="ysc")   # dequant scale out
            nc.vector.tensor_scalar(
                ysc[:], yab[:], 1.0 / 126.0, 1e-30,
                op0=mybir.AluOpType.mult, op1=mybir.AluOpType.max,
            )
            yqm = work.tile([128, ng], F32, tag="yqm")   # quant multiplier
            nc.vector.reciprocal(yqm[:], ysc[:])
            yq8 = work.tile([128, ng, E], mybir.dt.uint8, tag="eT")
            for g in range(ng):
                nc.vector.tensor_scalar(
                    yq8[:, g, :], yt[:, g, :], yqm[:, g:g + 1], 128.5,
                    op0=mybir.AluOpType.mult, op1=mybir.AluOpType.add,
                )
            nc.sync.dma_start(y_d[:].rearrange("(g p) e -> p g e", p=128), yq8[:])
            nc.sync.dma_start(ys_d[:], ysc[:])

    if hasattr(nc, "compile"):
        nc.compile()
    return nc


def shard_inputs(hidden_states, Wq, bq, Wk, bk, Wv, bv, Wo, bo, t=T):
    """Host-side sharding: returns in_maps for the 8 cores."""
    f32 = np.float32
    x = np.asarray(hidden_states, f32)
    Wq = np.asarray(Wq, f32) * SCALE
    bq = np.asarray(bq, f32) * SCALE
    ident = np.eye(64, dtype=NPBF16)
    ne = E // 128

    # per-batch x^T, int8-quantized per channel (row), E-sliced per core
    xT8, xsc = [], []
    for b in range(B):
        xb = np.ascontiguousarray(x[b, :t].T)                 # [E, t] f32
        am = np.maximum(np.abs(xb).max(axis=1, keepdims=True), 1e-30)
        xT8.append(np.rint(xb * (126.5 / am)).astype(np.int8))
        xsc.append(
            np.ascontiguousarray(
                (am[:, 0] / 126.5).reshape(E // 128, 128).T
            ).astype(f32)                                     # [128, ne]
        )

    # per-kv-head packed weight blob (shared by cores k and k+4)
    blobs = []
    for k in range(KVH):
        qsl = slice(k * G * D, (k + 1) * G * D)
        ksl = slice(k * D, (k + 1) * D)
        wq_l = np.ascontiguousarray(Wq[qsl].T).reshape(ne, 128, G * D)
        wq_l = np.ascontiguousarray(wq_l.transpose(1, 0, 2)).astype(NPBF16)
        wkv = np.concatenate(
            [np.asarray(Wk, f32)[ksl], np.asarray(Wv, f32)[ksl]], 0
        )
        wkv_l = np.ascontiguousarray(wkv.T).reshape(ne, 128, 2 * D)
        wkv_l = np.ascontiguousarray(wkv_l.transpose(1, 0, 2)).astype(NPBF16)
        wo_l = np.ascontiguousarray(np.asarray(Wo, f32)[:, qsl].T)    # [256,E]
        wo_l = np.ascontiguousarray(
            wo_l.reshape(2, 128, E).transpose(1, 0, 2)
        ).astype(NPBF16)
        blobs.append(
            np.concatenate([wq_l.ravel(), wkv_l.ravel(), wo_l.ravel()])
        )

    in_maps = []
    for cid in range(NCORES):
        b, k = cid // (NCORES // B), cid % (NCORES // B)
        qsl = slice(k * G * D, (k + 1) * G * D)
        ksl = slice(k * D, (k + 1) * D)
        bq_l = np.ascontiguousarray(bq[qsl].reshape(2, 128).T).astype(f32)
        bkv_l = np.concatenate(
            [np.asarray(bk, f32)[ksl], np.asarray(bv, f32)[ksl]]
        ).reshape(128, 1).astype(f32)
        half = WBLOB // 2
        in_maps.append({
            "xs": np.ascontiguousarray(xT8[b][k * (E // 4):(k + 1) * (E // 4)]),
            "xsc": xsc[b],
            "wh": np.ascontiguousarray(blobs[k][b * half:(b + 1) * half]),
            "bq": bq_l, "bkv": bkv_l, "ident": ident,
        })
    return in_maps


_last_res = None


def kernel(**inputs):
    global _last_res
    nc = build_nc(T)
    in_maps = shard_inputs(**inputs)
    res = run_bass_kernel_spmd(nc, in_maps, list(range(NCORES)))
    _last_res = res
    bo = np.asarray(inputs["bo"], np.float32)
    out = np.empty((B, T, E), np.float32)
    tq = T // 4
    for b in range(B):
        for j in range(4):
            u = np.asarray(res.results[b * 4 + j]["y"], np.float32)
            sc = np.asarray(res.results[b * 4 + j]["ys"], np.float32)
            rows = sc.T.reshape(tq, 1)     # scale for row g*128+p is sc[p, g]
            out[b, j * tq:(j + 1) * tq] = (u - 128.5) * rows + bo
    return out
